# revision 10
# baseline (speedup 1.0000x reference)
"""LocalLinOSS Trainium2 kernel — 8-core SPMD, sequence-sharded.

Model structure (reference): embedding lookup -> 4 sequential blocks; within a
timestep block i reads the running hidden h (reset to x_t each step), so the
only cross-time recurrence is the per-block diagonal state
    ns_t = coeff (.) ns_{t-1} + in_to_state @ LN(h_t).
This lets the model decompose into 4 sequential layer passes over the whole
sequence, each = big matmuls over L (parallel) + a first-order linear scan
(hardware tensor_tensor_scan), followed by the [L,D]@[D,V] output projection.

Sharding: L=4096 split into 8 chunks of T=512 (one per core). Per layer, each
core computes a local scan with zero initial state, AllGathers the 8 chunk
final states (1KB), combines them into its carry-in (precomputed decay powers
c^512 from the host), and re-runs the (cheap) scan with the correct initial
state. The output projection is computed on each core for its own T-chunk.

Layouts: h lives as [T=4x128 part, D free] (layernorm along free dim);
matmul operands live transposed [D or S part, T free]; PE transposes convert.
float32r (tf32-like, ~2^-11 rounding) feeds every matmul; h and the scan
state stay float32.
"""
import sys
sys.path.insert(0, "/opt/trn_rl_repo")
import numpy as np
import concourse.bass as bass
import concourse.bacc as bacc
import concourse.mybir as mybir
import concourse.tile as tile
from concourse.bass_utils import run_bass_kernel_spmd

L, D, S, NB, V = 4096, 256, 256, 4, 8000
NC = 8
T = L // NC            # 512 timesteps per core
P = 128
NT = T // P            # 4 T-tiles per core
NVC = 16               # projection V chunks
VC = V // NVC          # 500
f32 = mybir.dt.float32
f32r = mybir.dt.float32r
i32 = mybir.dt.int32
AF = mybir.ActivationFunctionType
OP = mybir.AluOpType

_cache = {}


def _build(use_outb: bool):
    if (nc_cached := _cache.get(use_outb)) is not None:
        return nc_cached
    nc = bacc.Bacc("TRN2", target_bir_lowering=False, debug=False,
                   enable_asserts=True, num_devices=NC)

    def din(name, shape, dtype=f32):
        return nc.dram_tensor(name, shape, dtype, kind="ExternalInput").ap()

    tok_idx = din("tok_idx", [P, NT], i32)
    tok_tab = din("tok_tab", [V, D])
    pos_pre = din("pos_pre", [P, NT * D])
    ident = din("ident", [P, P])
    ones8 = din("ones8", [8, 1])
    coef_in = din("coef_in", [P, NB * 2])          # coeff, col (i*2+st)
    wmat_in = din("wmat_in", [8, NB * S])          # per-core carry weights
    ubias_in = din("ubias_in", [P, NB * 2])        # W_in' @ ln_b
    dprime_in = din("dprime_in", [P, NB * 2])      # direct * ln_w
    dbias_in = din("dbias_in", [P, NB * 2])        # direct * ln_b
    opb_in = din("opb_in", [P, NB * 2])            # outp_b
    win_in = din("win_in", [NB, P, 4 * P])         # lhsT packs
    s2h_in = din("s2h_in", [NB, P, 4 * P])
    outp_in = din("outp_in", [NB, P, 4 * P])
    outwt_in = din("outwt_in", [2, P, V])
    outb_in = din("outb_in", [1, V])
    onesP_in = din("onesP_in", [1, P])
    out_d = nc.dram_tensor("out", [T, V], f32, kind="ExternalOutput").ap()

    with tile.TileContext(nc) as tc:
        with tc.tile_pool(name="const", bufs=1) as cst, \
             tc.tile_pool(name="wts", bufs=1) as wts, \
             tc.tile_pool(name="work", bufs=1) as wk, \
             tc.tile_pool(name="lay", bufs=2) as lay, \
             tc.tile_pool(name="psum", bufs=1, space="PSUM") as ps, \
             tc.tile_pool(name="stage", bufs=1) as stg, \
             tc.tile_pool(name="dram", bufs=1, space="DRAM") as dram:

            # ---- constant/weight loads ----
            id_r = cst.tile([P, P], f32r)
            nc.sync.dma_start(id_r[:], ident.bitcast(f32r))
            id_f = cst.tile([P, P], f32)
            nc.sync.dma_start(id_f[:], ident)
            ones8_sb = cst.tile([8, 1], f32)
            nc.sync.dma_start(ones8_sb[:], ones8)
            coef_sb = cst.tile([P, NB * 2], f32)
            nc.sync.dma_start(coef_sb[:], coef_in)
            wm_sb = cst.tile([8, NB * S], f32)
            nc.sync.dma_start(wm_sb[:], wmat_in)
            ub_sb = cst.tile([P, NB * 2], f32)
            nc.sync.dma_start(ub_sb[:], ubias_in)
            dp_sb = cst.tile([P, NB * 2], f32)
            nc.sync.dma_start(dp_sb[:], dprime_in)
            db_sb = cst.tile([P, NB * 2], f32)
            nc.sync.dma_start(db_sb[:], dbias_in)
            ob_sb = cst.tile([P, NB * 2], f32)
            nc.sync.dma_start(ob_sb[:], opb_in)
            win_sb = [wts.tile([P, 4 * P], f32r, name=f"win{i}") for i in range(NB)]
            s2h_sb = [wts.tile([P, 4 * P], f32r, name=f"s2h{i}") for i in range(NB)]
            outp_sb = [wts.tile([P, 4 * P], f32r, name=f"outp{i}") for i in range(NB)]
            for i in range(NB):
                nc.sync.dma_start(win_sb[i][:], win_in[i].bitcast(f32r))
                nc.sync.dma_start(s2h_sb[i][:], s2h_in[i].bitcast(f32r))
                nc.sync.dma_start(outp_sb[i][:], outp_in[i].bitcast(f32r))
            outwt_sb = [wts.tile([P, V], f32r, name=f"outwt{d}") for d in range(2)]
            for d in range(2):
                nc.sync.dma_start(outwt_sb[d][:], outwt_in[d].bitcast(f32r))
            if use_outb:
                outb_sb = cst.tile([1, V], f32r)
                nc.sync.dma_start(outb_sb[:], outb_in.bitcast(f32r))
                ones1_sb = cst.tile([1, P], f32r)
                nc.sync.dma_start(ones1_sb[:], onesP_in.bitcast(f32r))

            # ---- embedding gather + pos add ----
            ti_sb = wk.tile([P, NT], i32)
            nc.sync.dma_start(ti_sb[:], tok_idx)
            h = wk.tile([P, NT, D], f32)
            for ct in range(NT):
                nc.gpsimd.indirect_dma_start(
                    out=h[:, ct, :], out_offset=None, in_=tok_tab,
                    in_offset=bass.IndirectOffsetOnAxis(ap=ti_sb[:, ct:ct + 1], axis=0))
            pos_sb = wk.tile([P, NT * D], f32)
            nc.sync.dma_start(pos_sb[:], pos_pre)
            nc.vector.tensor_tensor(
                h[:].rearrange("p a b -> p (a b)"), h[:].rearrange("p a b -> p (a b)"),
                pos_sb[:], op=OP.add)

            hsT = [None, None]

            # ---- 4 sequential layer passes ----
            for i in range(NB):
                last = i == NB - 1
                # 1. layernorm stats + z = (h - mean) * rstd   (z in f32r)
                z = lay.tile([P, NT, D], f32r, tag="z")
                stats = lay.tile([P, NT, 6], f32, tag="stats")
                aggr = lay.tile([P, NT, 2], f32, tag="aggr")
                rstd = lay.tile([P, NT], f32, tag="rstd")
                for ct in range(NT):
                    nc.vector.bn_stats(stats[:, ct, :], h[:, ct, :])
                    nc.vector.bn_aggr(aggr[:, ct, :], stats[:, ct, :])
                    nc.vector.tensor_scalar_add(rstd[:, ct:ct + 1], aggr[:, ct, 1:2],
                                                1e-5)
                    nc.scalar.activation(rstd[:, ct:ct + 1], rstd[:, ct:ct + 1],
                                         AF.Sqrt)
                    nc.vector.reciprocal(rstd[:, ct:ct + 1], rstd[:, ct:ct + 1])
                    nc.vector.tensor_scalar(
                        z[:, ct, :], h[:, ct, :], aggr[:, ct, 0:1], rstd[:, ct:ct + 1],
                        op0=OP.subtract, op1=OP.mult)
                # 2. transpose z -> zT [D part, T free]
                zT = [lay.tile([P, T], f32r, tag=f"zT{d}", name=f"zT{d}") for d in range(2)]
                for d in range(2):
                    zt_ps = ps.tile([P, T], f32r, tag="pp", bufs=8, name=f"zt_ps{d}")
                    for ct in range(NT):
                        nc.tensor.transpose(zt_ps[:, ct * P:(ct + 1) * P],
                                            z[:, ct, d * P:(d + 1) * P], id_r[:])
                    nc.scalar.activation(zT[d][:], zt_ps[:], AF.Identity)
                # 3. u = W_in' @ z  (+ubias) ; scan
                u = [lay.tile([P, T], f32, tag=f"u{st}", name=f"u{st}") for st in range(2)]
                ns1 = lay.tile([P, 2], f32, tag="ns1")   # local-scan last states
                scr = lay.tile([P, T], f32, tag="scr")   # scratch for scan pass 1
                for st in range(2):
                    u_ps = ps.tile([P, T], f32, tag="pp", bufs=8, name=f"u_ps{st}")
                    for kt in range(2):
                        nc.tensor.matmul(u_ps[:], win_sb[i][:, (kt * 2 + st) * P:(kt * 2 + st + 1) * P],
                                         zT[kt][:], start=(kt == 0), stop=(kt == 1))
                    nc.scalar.activation(u[st][:], u_ps[:], AF.Identity,
                                         bias=ub_sb[:, i * 2 + st:i * 2 + st + 1])
                    cb = coef_sb[:, i * 2 + st:i * 2 + st + 1].to_broadcast((P, T))
                    nc.vector.tensor_tensor_scan(scr[:], cb, u[st][:], 0.0,
                                                 op0=OP.mult, op1=OP.add)
                    nc.vector.tensor_copy(ns1[:, st:st + 1], scr[:, T - 1:T])
                # 4. AllGather chunk-final states; combine into carry-in
                ag_in = dram.tile([2, P], f32, name=f"ag_in{i}")
                ag_out = dram.tile([NC, 2, P], f32, name=f"ag_out{i}",
                                   addr_space="Shared")
                for st in range(2):
                    nc.sync.dma_start(ag_in[st, :], ns1[:, st:st + 1])
                nc.gpsimd.collective_compute(
                    "AllGather", OP.bypass, replica_groups=[list(range(NC))],
                    ins=[ag_in[:]], outs=[ag_out[:]])
                gath = lay.tile([8, S], f32, tag="gath")
                nc.sync.dma_start(gath[:], ag_out[:].rearrange("c a b -> c (a b)"))
                q = lay.tile([8, S], f32, tag="q")
                nc.vector.tensor_tensor(q[:], wm_sb[:, i * S:(i + 1) * S], gath[:],
                                        op=OP.mult)
                # 5. corrected scan with carry initial
                ns = [lay.tile([P, T], f32r, tag=f"ns{st}", name=f"ns{st}") for st in range(2)]
                for st in range(2):
                    c_ps = ps.tile([P, 1], f32, tag="pp", bufs=8, name=f"c_ps{st}")
                    nc.tensor.matmul(c_ps[:], q[:, st * P:(st + 1) * P], ones8_sb[:],
                                     start=True, stop=True)
                    cb = coef_sb[:, i * 2 + st:i * 2 + st + 1].to_broadcast((P, T))
                    nc.vector.tensor_tensor_scan(ns[st][:], cb, u[st][:], c_ps[:, 0:1],
                                                 op0=OP.mult, op1=OP.add)
                # 6. mixed = gelu(s2h @ ns + dprime*z + dbias)
                mixed = [lay.tile([P, T], f32r, tag=f"mix{d}", name=f"mix{d}") for d in range(2)]
                gin = lay.tile([P, T], f32, tag="gin")
                for d in range(2):
                    m_ps = ps.tile([P, T], f32, tag="pp", bufs=8, name=f"m_ps{d}")
                    for st in range(2):
                        nc.tensor.matmul(m_ps[:], s2h_sb[i][:, (st * 2 + d) * P:(st * 2 + d + 1) * P],
                                         ns[st][:], start=(st == 0), stop=(st == 1))
                    nc.vector.scalar_tensor_tensor(
                        gin[:], zT[d][:].bitcast(f32),
                        dp_sb[:, i * 2 + d:i * 2 + d + 1], m_ps[:],
                        op0=OP.mult, op1=OP.add)
                    nc.scalar.activation(mixed[d][:], gin[:], AF.Gelu_apprx_tanh,
                                         bias=db_sb[:, i * 2 + d:i * 2 + d + 1])
                # 7. delta = outp_W' @ mixed (+outp_b)
                delta = [lay.tile([P, T], f32r, tag=f"del{d}", name=f"del{d}") for d in range(2)]
                for d2 in range(2):
                    d_ps = ps.tile([P, T], f32, tag="pp", bufs=8, name=f"d_ps{d2}")
                    for d in range(2):
                        nc.tensor.matmul(d_ps[:], outp_sb[i][:, (d * 2 + d2) * P:(d * 2 + d2 + 1) * P],
                                         mixed[d][:], start=(d == 0), stop=(d == 1))
                    nc.scalar.activation(delta[d2][:], d_ps[:], AF.Identity,
                                         bias=ob_sb[:, i * 2 + d2:i * 2 + d2 + 1])
                # 8. residual
                if not last:
                    # h += delta^T  (transpose delta back to [T part, D free])
                    for ct in range(NT):
                        dT_ps = ps.tile([P, D], f32r, tag="pp", bufs=8, name="dT_ps")
                        for d2 in range(2):
                            nc.tensor.transpose(dT_ps[:, d2 * P:(d2 + 1) * P],
                                                delta[d2][:, ct * P:(ct + 1) * P], id_r[:])
                        nc.vector.tensor_tensor(h[:, ct, :], h[:, ct, :],
                                                dT_ps[:].bitcast(f32), op=OP.add)
                else:
                    # hsT = h^T + delta  (stay in [D part, T free] for projection)
                    for d2 in range(2):
                        hT_ps = ps.tile([P, T], f32, tag="pp", bufs=8, name=f"hT_ps{d2}")
                        for ct in range(NT):
                            nc.tensor.transpose(hT_ps[:, ct * P:(ct + 1) * P],
                                                h[:, ct, d2 * P:(d2 + 1) * P], id_f[:])
                        hsT[d2] = wk.tile([P, T], f32r, name=f"hsT{d2}")
                        nc.vector.tensor_tensor(hsT[d2][:], delta[d2][:].bitcast(f32),
                                                hT_ps[:], op=OP.add)

            # ---- output projection: out[t, v] = hsT[:, t] . outwt[:, v] ----
            for mt in range(NT):
                for vc in range(NVC):
                    p_ps = ps.tile([P, VC], f32, tag="pp", bufs=8, name="p_ps")
                    for d in range(2):
                        nc.tensor.matmul(p_ps[:], hsT[d][:, mt * P:(mt + 1) * P],
                                         outwt_sb[d][:, vc * VC:(vc + 1) * VC],
                                         start=(d == 0),
                                         stop=(d == 1 and not use_outb))
                    if use_outb:
                        nc.tensor.matmul(p_ps[:], ones1_sb[:],
                                         outb_sb[:, vc * VC:(vc + 1) * VC],
                                         start=False, stop=True)
                    st_t = stg.tile([P, VC], f32, tag="stg", bufs=8)
                    if vc % 2 == 0:
                        nc.vector.tensor_copy(st_t[:], p_ps[:])
                    else:
                        nc.scalar.activation(st_t[:], p_ps[:], AF.Identity)
                    nc.sync.dma_start(
                        out_d[mt * P:(mt + 1) * P, vc * VC:(vc + 1) * VC], st_t[:])

    nc.compile()
    _cache[use_outb] = nc
    return nc


def _pack_lhsT(w):
    """w: [M, K] weight for out = w @ x. Returns [128, (K/128)*(M/128)*128] lhsT pack;
    block b = kt*nmt + mt holds lhsT[kt*128+p, mt*128+m]."""
    M, K = w.shape
    lhsT = np.ascontiguousarray(w.T)                       # [K, M]
    t = lhsT.reshape(K // P, P, M // P, P)                 # [kt, p, mt, m]
    return np.ascontiguousarray(t.transpose(1, 0, 2, 3).reshape(P, -1))


def kernel(**inputs):
    xs = {k: np.asarray(v) for k, v in inputs.items()}
    tokens = xs["tokens"].astype(np.int32)
    token_embed = xs["token_embed"].astype(np.float32)
    pos_embed = xs["pos_embed"].astype(np.float32)
    in_to_state = xs["in_to_state"].astype(np.float64)
    state_to_hidden = xs["state_to_hidden"].astype(np.float64)
    direct = xs["direct"].astype(np.float64)
    a_diag = xs["a_diag"].astype(np.float64)
    g_diag = xs["g_diag"].astype(np.float64)
    dtp = xs["dt"].astype(np.float64)
    ln_w = xs["ln_w"].astype(np.float64)
    ln_b = xs["ln_b"].astype(np.float64)
    outp_W = xs["outp_W"].astype(np.float64)
    outp_b = xs["outp_b"].astype(np.float32)
    out_W = xs["out_W"].astype(np.float32)
    out_b = xs["out_b"].astype(np.float32)

    def softplus(x):
        return np.logaddexp(0.0, x)

    dt_e = softplus(dtp) + 1e-4
    coeff = np.exp(-softplus(g_diag) * dt_e) * np.cos(a_diag * dt_e)   # [NB, S]
    cdecay = coeff ** T                                                 # [NB, S]

    # packed weights (shared across cores)
    win_pack = np.stack([_pack_lhsT(in_to_state[i] * ln_w[i][None, :]) for i in range(NB)]).astype(np.float32)
    s2h_pack = np.stack([_pack_lhsT(state_to_hidden[i]) for i in range(NB)]).astype(np.float32)
    outp_pack = np.stack([_pack_lhsT(outp_W[i]) for i in range(NB)]).astype(np.float32)
    outwt_pack = np.ascontiguousarray(out_W.T.reshape(2, P, V)).astype(np.float32)
    ubias = np.stack([in_to_state[i] @ ln_b[i] for i in range(NB)])     # [NB, S]
    dprime = direct * ln_w                                              # [NB, D]
    dbias = direct * ln_b                                               # [NB, D]

    def cols(v):  # [NB, 256] -> [128, NB*2] with col (i*2+half)
        return np.ascontiguousarray(
            v.reshape(NB, 2, P).transpose(2, 0, 1).reshape(P, NB * 2)).astype(np.float32)

    coef_cols = cols(coeff)
    ubias_cols = cols(ubias)
    dprime_cols = cols(dprime)
    dbias_cols = cols(dbias)
    opb_cols = cols(np.broadcast_to(outp_b, (NB, D)).astype(np.float64))
    use_outb = bool(np.any(out_b != 0.0))

    base = dict(
        tok_tab=token_embed, ident=np.eye(P, dtype=np.float32),
        ones8=np.ones((8, 1), np.float32), coef_in=coef_cols,
        ubias_in=ubias_cols, dprime_in=dprime_cols, dbias_in=dbias_cols,
        opb_in=opb_cols, win_in=win_pack, s2h_in=s2h_pack, outp_in=outp_pack,
        outwt_in=outwt_pack, outb_in=out_b.reshape(1, V),
        onesP_in=np.ones((1, P), np.float32),
    )

    in_maps = []
    for k in range(NC):
        sl = slice(k * T, (k + 1) * T)
        tk = tokens[sl].reshape(NT, P).T.copy()            # [128, NT]
        pos = np.ascontiguousarray(
            pos_embed[sl].reshape(NT, P, D).transpose(1, 0, 2).reshape(P, NT * D))
        # carry weights: wmat[j, s] = cdecay[s]^(k-1-j) for j<k else 0
        wm = np.zeros((8, NB, S), np.float64)
        for j in range(k):
            wm[j] = cdecay ** (k - 1 - j)
        wm = wm.reshape(8, NB * S).astype(np.float32)
        in_maps.append(dict(base, tok_idx=tk, pos_pre=pos, wmat_in=wm))

    nc = _build(use_outb)
    import os
    trace = bool(os.environ.get("BASS_KERNEL_TRACE"))
    res = run_bass_kernel_spmd(nc, in_maps, core_ids=list(range(NC)), trace=trace)
    if trace:
        kernel.last_exec_time_ns = res.exec_time_ns
        kernel.last_results = res
    return np.concatenate([res.results[k]["out"] for k in range(NC)], axis=0)


# revision 13
# speedup vs baseline: 1.0558x; 1.0558x over previous
"""LocalLinOSS Trainium2 kernel — 8-core SPMD, sequence-sharded.

Model structure (reference): embedding lookup -> 4 sequential blocks; within a
timestep, block i reads the running hidden h (reset to x_t each step), so the
only cross-time recurrence is the per-block diagonal state
    ns_t = coeff (.) ns_{t-1} + in_to_state @ LN(h_t).
The model therefore decomposes into 4 sequential layer passes over the whole
sequence, each = big matmuls over L (parallel) + a first-order linear scan
(hardware tensor_tensor_scan), followed by the [L,D]@[D,V] output projection.

Sharding: L=4096 split into 8 chunks of T=512 (one per core). Per layer, each
core computes a local scan with zero initial state, AllGathers the 8 chunk
final states (1KB payload), combines them into its carry-in using
host-precomputed decay powers (cdecay^j), and applies the carry as a
correction through the next matmul: s2h @ ns = s2h @ ns_local + s2h @ A where
A[s,t] = c_s^{t+1} * carry_s. The two s2h @ ns_local matmuls are issued before
the collective so they overlap it. The output projection runs per-core on its
own T-chunk in bf16 (fp32 accumulate).

Layouts: h lives as [T=4x128 part, D free] (layernorm along free dim);
matmul operands live transposed [D or S part, T free]; PE transposes convert.
float32r (tf32-like, ~2^-11 rounding at write) feeds the backbone matmuls;
h and the scan state stay float32.
"""
import os
import sys
sys.path.insert(0, "/opt/trn_rl_repo")
import numpy as np
import concourse.bass as bass
import concourse.bacc as bacc
import concourse.mybir as mybir
import concourse.tile as tile
from concourse.bass_utils import run_bass_kernel_spmd

L, D, S, NB, V = 4096, 256, 256, 4, 8000
NC = 8
T = L // NC            # 512 timesteps per core
P = 128
NT = T // P            # 4 T-tiles per core
NVC = 16               # projection V chunks
VC = V // NVC          # 500
f32 = mybir.dt.float32
f32r = mybir.dt.float32r
bf16 = mybir.dt.bfloat16
i32 = mybir.dt.int32
AF = mybir.ActivationFunctionType
OP = mybir.AluOpType

_cache = {}


def _build(use_outb: bool):
    if (nc_cached := _cache.get(use_outb)) is not None:
        return nc_cached
    nc = bacc.Bacc("TRN2", target_bir_lowering=False, debug=False,
                   enable_asserts=True, num_devices=NC)

    def din(name, shape, dtype=f32):
        return nc.dram_tensor(name, shape, dtype, kind="ExternalInput").ap()

    tok_idx = din("tok_idx", [P, NT], i32)
    tok_tab = din("tok_tab", [V, D])
    pos_pre = din("pos_pre", [P, NT * D])
    ident = din("ident", [P, P])
    ones8 = din("ones8", [8, 1])
    onesT = din("onesT", [1, T], bf16)
    coef_in = din("coef_in", [P, NB * 2])          # coeff, col (i*2+st)
    wmat_in = din("wmat_in", [8, NB * S])          # per-core carry weights
    ubias_in = din("ubias_in", [1, NB * 2 * P], bf16)  # W_in' @ ln_b lhsT rows
    dprime_in = din("dprime_in", [P, NB * 2])      # direct * ln_w
    dbias_in = din("dbias_in", [P, NB * 2])        # direct * ln_b
    opb_in = din("opb_in", [P, NB * 2])            # outp_b
    win_in = din("win_in", [NB, P, 4 * P])         # lhsT packs
    s2h_in = din("s2h_in", [NB, P, 4 * P])
    outp_in = din("outp_in", [NB, P, 4 * P])
    cpow_in = din("cpow_in", [NB, 2, P, T])        # c^(t+1)
    outwt_in = din("outwt_in", [2, P, V], bf16)
    outb_in = din("outb_in", [1, V], bf16)
    out_d = nc.dram_tensor("out", [T, V], f32, kind="ExternalOutput").ap()

    with tile.TileContext(nc) as tc:
        with tc.tile_pool(name="const", bufs=1) as cst, \
             tc.tile_pool(name="wts", bufs=1) as wts, \
             tc.tile_pool(name="work", bufs=1) as wk, \
             tc.tile_pool(name="lay", bufs=2) as lay, \
             tc.tile_pool(name="psum", bufs=1, space="PSUM") as ps, \
             tc.tile_pool(name="stage", bufs=1) as stg, \
             tc.tile_pool(name="dram", bufs=1, space="DRAM") as dram:

            # ---- loads, ordered by when they are first needed ----
            ti_sb = wk.tile([P, NT], i32)
            nc.sync.dma_start(ti_sb[:], tok_idx)
            id_r = cst.tile([P, P], f32r)
            nc.sync.dma_start(id_r[:], ident.bitcast(f32r))
            id_f = cst.tile([P, P], f32)
            nc.sync.dma_start(id_f[:], ident)
            pos_sb = wk.tile([P, NT * D], f32)
            nc.sync.dma_start(pos_sb[:], pos_pre)
            coef_sb = cst.tile([P, NB * 2], f32)
            nc.sync.dma_start(coef_sb[:], coef_in)
            ones8_sb = cst.tile([8, 1], f32)
            nc.sync.dma_start(ones8_sb[:], ones8)
            onesT_sb = cst.tile([1, T], bf16)
            nc.sync.dma_start(onesT_sb[:], onesT)
            wm_sb = cst.tile([8, NB * S], f32)
            nc.sync.dma_start(wm_sb[:], wmat_in)
            ub_sb = cst.tile([1, NB * 2 * P], bf16)
            nc.sync.dma_start(ub_sb[:], ubias_in)
            dp_sb = cst.tile([P, NB * 2], f32)
            nc.sync.dma_start(dp_sb[:], dprime_in)
            db_sb = cst.tile([P, NB * 2], f32)
            nc.sync.dma_start(db_sb[:], dbias_in)
            ob_sb = cst.tile([P, NB * 2], f32)
            nc.sync.dma_start(ob_sb[:], opb_in)
            scrap = cst.tile([1, 1], f32)          # dummy act target
            # per-layer weights, in layer order so layer 0 unblocks first
            win_sb, s2h_sb, outp_sb, cpow_sb = [], [], [], []
            for i in range(NB):
                win_sb.append(wts.tile([P, 4 * P], f32r, name=f"win{i}"))
                nc.sync.dma_start(win_sb[i][:], win_in[i].bitcast(f32r))
                s2h_sb.append(wts.tile([P, 4 * P], f32r, name=f"s2h{i}"))
                nc.sync.dma_start(s2h_sb[i][:], s2h_in[i].bitcast(f32r))
                outp_sb.append(wts.tile([P, 4 * P], f32r, name=f"outp{i}"))
                nc.sync.dma_start(outp_sb[i][:], outp_in[i].bitcast(f32r))
                cpow_sb.append(wts.tile([P, 2, T], f32, name=f"cpow{i}"))
                nc.sync.dma_start(cpow_sb[i][:], cpow_in[i].rearrange("a p t -> p a t"))
            # projection weights last — only needed after the backbone
            outwt_sb = [wts.tile([P, V], bf16, name=f"outwt{d}") for d in range(2)]
            for d in range(2):
                nc.sync.dma_start(outwt_sb[d][:], outwt_in[d])
            if use_outb:
                outb_sb = cst.tile([1, V], bf16)
                nc.sync.dma_start(outb_sb[:], outb_in)
                ones1_sb = cst.tile([1, P], bf16)
                nc.sync.dma_start(ones1_sb[:], onesT[:, :P])

            # ---- embedding gather + pos add ----
            h = wk.tile([P, NT, D], f32)
            for ct in range(NT):
                nc.gpsimd.indirect_dma_start(
                    out=h[:, ct, :], out_offset=None, in_=tok_tab,
                    in_offset=bass.IndirectOffsetOnAxis(ap=ti_sb[:, ct:ct + 1], axis=0))
            nc.vector.tensor_tensor(
                h[:].rearrange("p a b -> p (a b)"), h[:].rearrange("p a b -> p (a b)"),
                pos_sb[:], op=OP.add)

            hsT = [None, None]

            # ---- 4 sequential layer passes ----
            for i in range(NB):
                last = i == NB - 1
                # 1. layernorm stats + z = (h - mean) * rstd   (z in f32r)
                z = lay.tile([P, NT, D], f32r, tag="z")
                stats = lay.tile([P, NT, 6], f32, tag="stats")
                aggr = lay.tile([P, NT, 2], f32, tag="aggr")
                rstd = lay.tile([P, NT], f32, tag="rstd")
                for ct in range(NT):
                    nc.vector.bn_stats(stats[:, ct, :], h[:, ct, :])
                    nc.vector.bn_aggr(aggr[:, ct, :], stats[:, ct, :])
                nc.vector.tensor_scalar_add(rstd[:], aggr[:, :, 1], 1e-5)
                nc.scalar.activation(rstd[:], rstd[:], AF.Sqrt)
                # prefetch the gelu table while the LN chain continues on DVE
                nc.scalar.activation(scrap[:], scrap[:], AF.Gelu_apprx_tanh)
                nc.vector.reciprocal(rstd[:], rstd[:])
                for ct in range(NT):
                    nc.vector.tensor_scalar(
                        z[:, ct, :], h[:, ct, :], aggr[:, ct, 0:1], rstd[:, ct:ct + 1],
                        op0=OP.subtract, op1=OP.mult)
                # 2. transpose z -> zT [D part, T free]
                zT = [lay.tile([P, T], f32r, tag=f"zT{d}", name=f"zT{d}") for d in range(2)]
                for d in range(2):
                    zt_ps = ps.tile([P, T], f32r, tag="pp", bufs=8, name=f"zt_ps{d}")
                    for ct in range(NT):
                        nc.tensor.transpose(zt_ps[:, ct * P:(ct + 1) * P],
                                            z[:, ct, d * P:(d + 1) * P], id_r[:])
                    nc.scalar.activation(zT[d][:], zt_ps[:], AF.Identity)
                # 3. u = W_in' @ z (+ubias via ones-row matmul); local scan from PSUM
                ns1 = [lay.tile([P, T], f32r, tag=f"ns1{st}", name=f"ns1{st}")
                       for st in range(2)]
                last2 = lay.tile([P, 2], f32r, tag="last2")
                u_ps = [None, None]
                for st in range(2):
                    u_ps[st] = ps.tile([P, T], f32, tag="pp", bufs=8, name=f"u_ps{st}")
                    for kt in range(2):
                        nc.tensor.matmul(u_ps[st][:],
                                         win_sb[i][:, (kt * 2 + st) * P:(kt * 2 + st + 1) * P],
                                         zT[kt][:], start=(kt == 0), stop=False)
                    nc.tensor.matmul(u_ps[st][:],
                                     ub_sb[:, (i * 2 + st) * P:(i * 2 + st + 1) * P],
                                     onesT_sb[:], start=False, stop=True)
                    cb = coef_sb[:, i * 2 + st:i * 2 + st + 1].to_broadcast((P, T))
                    nc.vector.tensor_tensor_scan(ns1[st][:], cb, u_ps[st][:], 0.0,
                                                 op0=OP.mult, op1=OP.add)
                    nc.vector.tensor_copy(last2[:, st:st + 1], ns1[st][:, T - 1:T])
                # 4. export chunk-final states (transposed: 2 descriptors),
                #    AllGather, combine into carry
                lt_ps = ps.tile([2, P], f32r, tag="pp", bufs=8, name="lt_ps")
                nc.tensor.transpose(lt_ps[:], last2[:], id_r[:])
                exp_sb = lay.tile([2, P], f32, tag="exp")
                nc.scalar.activation(exp_sb[:], lt_ps[:], AF.Identity)
                ag_in = dram.tile([2, P], f32, name=f"ag_in{i}")
                ag_out = dram.tile([NC, 2, P], f32, name=f"ag_out{i}",
                                   addr_space="Shared")
                nc.sync.dma_start(ag_in[:], exp_sb[:])
                nc.gpsimd.collective_compute(
                    "AllGather", OP.bypass, replica_groups=[list(range(NC))],
                    ins=[ag_in[:]], outs=[ag_out[:]])
                gath = lay.tile([8, S], f32, tag="gath")
                nc.sync.dma_start(gath[:], ag_out[:].rearrange("c a b -> c (a b)"))
                q = lay.tile([8, S], f32, tag="q")
                nc.vector.tensor_tensor(q[:], wm_sb[:, i * S:(i + 1) * S], gath[:],
                                        op=OP.mult)
                c_ps = [None, None]
                A = [lay.tile([P, T], f32r, tag=f"A{st}", name=f"A{st}")
                     for st in range(2)]
                for st in range(2):
                    c_ps[st] = ps.tile([P, 1], f32, tag="pp", bufs=8, name=f"c_ps{st}")
                    nc.tensor.matmul(c_ps[st][:], q[:, st * P:(st + 1) * P],
                                     ones8_sb[:], start=True, stop=True)
                    nc.vector.tensor_scalar_mul(A[st][:], cpow_sb[i][:, st, :],
                                                c_ps[st][:, 0:1])
                # 5. mixed = gelu(s2h @ (ns1 + A) + dprime*z + dbias)
                #    (the ns1 matmuls are issued first: they overlap the collective)
                mixed = [lay.tile([P, T], f32r, tag=f"mix{d}", name=f"mix{d}")
                         for d in range(2)]
                gin = lay.tile([P, T], f32, tag="gin")
                m_ps = [None, None]
                for d in range(2):
                    m_ps[d] = ps.tile([P, T], f32, tag="pp", bufs=8, name=f"m_ps{d}")
                    for st in range(2):
                        nc.tensor.matmul(m_ps[d][:],
                                         s2h_sb[i][:, (st * 2 + d) * P:(st * 2 + d + 1) * P],
                                         ns1[st][:], start=(st == 0), stop=False)
                for d in range(2):
                    for st in range(2):
                        nc.tensor.matmul(m_ps[d][:],
                                         s2h_sb[i][:, (st * 2 + d) * P:(st * 2 + d + 1) * P],
                                         A[st][:], start=False, stop=(st == 1))
                    nc.vector.scalar_tensor_tensor(
                        gin[:], zT[d][:].bitcast(f32),
                        dp_sb[:, i * 2 + d:i * 2 + d + 1], m_ps[d][:],
                        op0=OP.mult, op1=OP.add)
                    nc.scalar.activation(mixed[d][:], gin[:], AF.Gelu_apprx_tanh,
                                         bias=db_sb[:, i * 2 + d:i * 2 + d + 1])
                # prefetch sqrt table for the next layer's LN during the tail
                if not last:
                    nc.scalar.activation(scrap[:], scrap[:], AF.Sqrt)
                # 6. delta = outp_W' @ mixed (+outp_b)
                delta = [lay.tile([P, T], f32r, tag=f"del{d}", name=f"del{d}")
                         for d in range(2)]
                for d2 in range(2):
                    d_ps = ps.tile([P, T], f32, tag="pp", bufs=8, name=f"d_ps{d2}")
                    for d in range(2):
                        nc.tensor.matmul(d_ps[:],
                                         outp_sb[i][:, (d * 2 + d2) * P:(d * 2 + d2 + 1) * P],
                                         mixed[d][:], start=(d == 0), stop=(d == 1))
                    nc.scalar.activation(delta[d2][:], d_ps[:], AF.Identity,
                                         bias=ob_sb[:, i * 2 + d2:i * 2 + d2 + 1])
                # 7. residual
                if not last:
                    for ct in range(NT):
                        dT_ps = ps.tile([P, D], f32r, tag="pp", bufs=8, name="dT_ps")
                        for d2 in range(2):
                            nc.tensor.transpose(dT_ps[:, d2 * P:(d2 + 1) * P],
                                                delta[d2][:, ct * P:(ct + 1) * P], id_r[:])
                        nc.vector.tensor_tensor(h[:, ct, :], h[:, ct, :],
                                                dT_ps[:].bitcast(f32), op=OP.add)
                else:
                    # hsT = h^T + delta in [D part, T free], bf16 for projection
                    for d2 in range(2):
                        hT_ps = ps.tile([P, T], f32, tag="pp", bufs=8, name=f"hT_ps{d2}")
                        for ct in range(NT):
                            nc.tensor.transpose(hT_ps[:, ct * P:(ct + 1) * P],
                                                h[:, ct, d2 * P:(d2 + 1) * P], id_f[:])
                        hsT[d2] = wk.tile([P, T], bf16, name=f"hsT{d2}")
                        nc.vector.tensor_tensor(hsT[d2][:], delta[d2][:].bitcast(f32),
                                                hT_ps[:], op=OP.add)

            # ---- output projection: out[t, v] = hsT[:, t] . outwt[:, v] ----
            for mt in range(NT):
                for vc in range(NVC):
                    p_ps = ps.tile([P, VC], f32, tag="pp", bufs=8, name="p_ps")
                    for d in range(2):
                        nc.tensor.matmul(p_ps[:], hsT[d][:, mt * P:(mt + 1) * P],
                                         outwt_sb[d][:, vc * VC:(vc + 1) * VC],
                                         start=(d == 0),
                                         stop=(d == 1 and not use_outb))
                    if use_outb:
                        nc.tensor.matmul(p_ps[:], ones1_sb[:],
                                         outb_sb[:, vc * VC:(vc + 1) * VC],
                                         start=False, stop=True)
                    st_t = stg.tile([P, VC], f32, tag="stg", bufs=8)
                    if vc % 2 == 0:
                        nc.vector.tensor_copy(st_t[:], p_ps[:])
                    else:
                        nc.scalar.activation(st_t[:], p_ps[:], AF.Identity)
                    nc.sync.dma_start(
                        out_d[mt * P:(mt + 1) * P, vc * VC:(vc + 1) * VC], st_t[:])

    nc.compile()
    _cache[use_outb] = nc
    return nc


def _pack_lhsT(w):
    """w: [M, K] weight for out = w @ x. Returns [128, (K/128)*(M/128)*128] lhsT pack;
    block b = kt*nmt + mt holds lhsT[kt*128+p, mt*128+m]."""
    M, K = w.shape
    lhsT = np.ascontiguousarray(w.T)                       # [K, M]
    t = lhsT.reshape(K // P, P, M // P, P)                 # [kt, p, mt, m]
    return np.ascontiguousarray(t.transpose(1, 0, 2, 3).reshape(P, -1))


def kernel(**inputs):
    xs = {k: np.asarray(v) for k, v in inputs.items()}
    tokens = xs["tokens"].astype(np.int32)
    token_embed = xs["token_embed"].astype(np.float32)
    pos_embed = xs["pos_embed"].astype(np.float32)
    in_to_state = xs["in_to_state"].astype(np.float64)
    state_to_hidden = xs["state_to_hidden"].astype(np.float64)
    direct = xs["direct"].astype(np.float64)
    a_diag = xs["a_diag"].astype(np.float64)
    g_diag = xs["g_diag"].astype(np.float64)
    dtp = xs["dt"].astype(np.float64)
    ln_w = xs["ln_w"].astype(np.float64)
    ln_b = xs["ln_b"].astype(np.float64)
    outp_W = xs["outp_W"].astype(np.float64)
    outp_b = xs["outp_b"].astype(np.float32)
    out_W = xs["out_W"].astype(np.float32)
    out_b = xs["out_b"].astype(np.float32)

    def softplus(x):
        return np.logaddexp(0.0, x)

    dt_e = softplus(dtp) + 1e-4
    coeff = np.exp(-softplus(g_diag) * dt_e) * np.cos(a_diag * dt_e)   # [NB, S]
    cdecay = coeff ** T                                                 # [NB, S]
    # c^(t+1) tables for the carry correction, [NB, 2, P, T]
    tpow = np.arange(1, T + 1, dtype=np.float64)
    cpow = coeff.reshape(NB, 2, P, 1) ** tpow.reshape(1, 1, 1, T)

    # packed weights (shared across cores)
    win_pack = np.stack([_pack_lhsT(in_to_state[i] * ln_w[i][None, :]) for i in range(NB)]).astype(np.float32)
    s2h_pack = np.stack([_pack_lhsT(state_to_hidden[i]) for i in range(NB)]).astype(np.float32)
    outp_pack = np.stack([_pack_lhsT(outp_W[i]) for i in range(NB)]).astype(np.float32)
    outwt_pack = np.ascontiguousarray(out_W.T.reshape(2, P, V))
    import ml_dtypes
    outwt_bf16 = outwt_pack.astype(ml_dtypes.bfloat16)
    ubias = np.stack([in_to_state[i] @ ln_b[i] for i in range(NB)])     # [NB, S]
    dprime = direct * ln_w                                              # [NB, D]
    dbias = direct * ln_b                                               # [NB, D]

    def cols(v):  # [NB, 256] -> [128, NB*2] with col (i*2+half)
        return np.ascontiguousarray(
            v.reshape(NB, 2, P).transpose(2, 0, 1).reshape(P, NB * 2)).astype(np.float32)

    use_outb = bool(np.any(out_b != 0.0))

    base = dict(
        tok_tab=token_embed, ident=np.eye(P, dtype=np.float32),
        ones8=np.ones((8, 1), np.float32),
        onesT=np.ones((1, T), ml_dtypes.bfloat16),
        coef_in=cols(coeff),
        ubias_in=ubias.reshape(1, NB * 2 * P).astype(ml_dtypes.bfloat16),
        dprime_in=cols(dprime), dbias_in=cols(dbias),
        opb_in=cols(np.broadcast_to(outp_b, (NB, D)).astype(np.float64)),
        win_in=win_pack, s2h_in=s2h_pack, outp_in=outp_pack,
        cpow_in=cpow.astype(np.float32),
        outwt_in=outwt_bf16, outb_in=out_b.reshape(1, V).astype(ml_dtypes.bfloat16),
    )

    in_maps = []
    for k in range(NC):
        sl = slice(k * T, (k + 1) * T)
        tk = tokens[sl].reshape(NT, P).T.copy()            # [128, NT]
        pos = np.ascontiguousarray(
            pos_embed[sl].reshape(NT, P, D).transpose(1, 0, 2).reshape(P, NT * D))
        # carry weights: wmat[j, s] = cdecay[s]^(k-1-j) for j<k else 0
        wm = np.zeros((8, NB, S), np.float64)
        for j in range(k):
            wm[j] = cdecay ** (k - 1 - j)
        wm = wm.reshape(8, NB * S).astype(np.float32)
        in_maps.append(dict(base, tok_idx=tk, pos_pre=pos, wmat_in=wm))

    nc = _build(use_outb)
    trace = bool(os.environ.get("BASS_KERNEL_TRACE"))
    res = run_bass_kernel_spmd(nc, in_maps, core_ids=list(range(NC)), trace=trace)
    if trace:
        kernel.last_exec_time_ns = res.exec_time_ns
        kernel.last_results = res
    return np.concatenate([res.results[k]["out"] for k in range(NC)], axis=0)


# revision 14
# speedup vs baseline: 1.1120x; 1.0532x over previous
"""LocalLinOSS Trainium2 kernel — 8-core SPMD, sequence-sharded.

Model structure (reference): embedding lookup -> 4 sequential blocks; within a
timestep, block i reads the running hidden h (reset to x_t each step), so the
only cross-time recurrence is the per-block diagonal state
    ns_t = coeff (.) ns_{t-1} + in_to_state @ LN(h_t).
The model therefore decomposes into 4 sequential layer passes over the whole
sequence, each = big matmuls over L (parallel) + a first-order linear scan
(hardware tensor_tensor_scan), followed by the [L,D]@[D,V] output projection.

Sharding: L=4096 split into 8 chunks of T=512 (one per core). Per layer, each
core computes a local scan with zero initial state, AllGathers the 8 chunk
final states (1KB payload), combines them into its carry-in using
host-precomputed decay powers (cdecay^j), and applies the carry as a
correction through the next matmul: s2h @ ns = s2h @ ns_local + s2h @ A where
A[s,t] = c_s^{t+1} * carry_s. The two s2h @ ns_local matmuls are issued before
the collective so they overlap it. The output projection runs per-core on its
own T-chunk in bf16 (fp32 accumulate).

Layouts: h lives as [T=4x128 part, D free] (layernorm along free dim);
matmul operands live transposed [D or S part, T free]; PE transposes convert.
float32r (tf32-like, ~2^-11 rounding at write) feeds the backbone matmuls;
h and the scan state stay float32.
"""
import os
import sys
sys.path.insert(0, "/opt/trn_rl_repo")
import numpy as np
import concourse.bass as bass
import concourse.bacc as bacc
import concourse.mybir as mybir
import concourse.tile as tile
from concourse.bass_utils import run_bass_kernel_spmd

L, D, S, NB, V = 4096, 256, 256, 4, 8000
NC = 8
T = L // NC            # 512 timesteps per core
P = 128
NT = T // P            # 4 T-tiles per core
NVC = 16               # projection V chunks
VC = V // NVC          # 500
f32 = mybir.dt.float32
f32r = mybir.dt.float32r
bf16 = mybir.dt.bfloat16
i32 = mybir.dt.int32
AF = mybir.ActivationFunctionType
OP = mybir.AluOpType

_cache = {}


def _build(use_outb: bool):
    if (nc_cached := _cache.get(use_outb)) is not None:
        return nc_cached
    nc = bacc.Bacc("TRN2", target_bir_lowering=False, debug=False,
                   enable_asserts=True, num_devices=NC)

    def din(name, shape, dtype=f32):
        return nc.dram_tensor(name, shape, dtype, kind="ExternalInput").ap()

    tok_idx = din("tok_idx", [P, NT], i32)
    tok_tab = din("tok_tab", [V, D])
    pos_pre = din("pos_pre", [P, NT * D])
    ident = din("ident", [P, P])
    ones8 = din("ones8", [8, 1])
    onesT = din("onesT", [1, T], bf16)
    coef_in = din("coef_in", [P, NB * 2])          # coeff, col (i*2+st)
    wmat_in = din("wmat_in", [8, NB * S])          # per-core carry weights
    ubias_in = din("ubias_in", [1, NB * 2 * P], bf16)  # W_in' @ ln_b lhsT rows
    dprime_in = din("dprime_in", [P, NB * 2])      # direct * ln_w
    dbias_in = din("dbias_in", [P, NB * 2])        # direct * ln_b
    opb_in = din("opb_in", [P, NB * 2])            # outp_b
    win_in = din("win_in", [NB, P, 4 * P])         # lhsT packs
    s2h_in = din("s2h_in", [NB, P, 4 * P])
    outp_in = din("outp_in", [NB, P, 4 * P])
    cpow_in = din("cpow_in", [NB, 2, P, T])        # c^(t+1)
    outwt_in = din("outwt_in", [2, P, V], bf16)
    outb_in = din("outb_in", [1, V], bf16)
    out_d = nc.dram_tensor("out", [T, V], f32, kind="ExternalOutput").ap()

    with tile.TileContext(nc) as tc:
        with tc.tile_pool(name="const", bufs=1) as cst, \
             tc.tile_pool(name="wts", bufs=1) as wts, \
             tc.tile_pool(name="work", bufs=1) as wk, \
             tc.tile_pool(name="lay", bufs=2) as lay, \
             tc.tile_pool(name="psum", bufs=1, space="PSUM") as ps, \
             tc.tile_pool(name="stage", bufs=1) as stg, \
             tc.tile_pool(name="dram", bufs=1, space="DRAM") as dram:

            # ---- loads, ordered by when they are first needed ----
            ti_sb = wk.tile([P, NT], i32)
            nc.sync.dma_start(ti_sb[:], tok_idx)
            id_r = cst.tile([P, P], f32r)
            nc.sync.dma_start(id_r[:], ident.bitcast(f32r))
            id_f = cst.tile([P, P], f32)
            nc.sync.dma_start(id_f[:], ident)
            pos_sb = wk.tile([P, NT * D], f32)
            nc.sync.dma_start(pos_sb[:], pos_pre)
            coef_sb = cst.tile([P, NB * 2], f32)
            nc.sync.dma_start(coef_sb[:], coef_in)
            ones8_sb = cst.tile([8, 1], f32)
            nc.sync.dma_start(ones8_sb[:], ones8)
            onesT_sb = cst.tile([1, T], bf16)
            nc.sync.dma_start(onesT_sb[:], onesT)
            wm_sb = cst.tile([8, NB * S], f32)
            nc.sync.dma_start(wm_sb[:], wmat_in)
            ub_sb = cst.tile([1, NB * 2 * P], bf16)
            nc.sync.dma_start(ub_sb[:], ubias_in)
            dp_sb = cst.tile([P, NB * 2], f32)
            nc.sync.dma_start(dp_sb[:], dprime_in)
            db_sb = cst.tile([P, NB * 2], f32)
            nc.sync.dma_start(db_sb[:], dbias_in)
            ob_sb = cst.tile([P, NB * 2], f32)
            nc.sync.dma_start(ob_sb[:], opb_in)
            scrap = cst.tile([1, 1], f32)          # dummy act target
            # per-layer weights, in layer order so layer 0 unblocks first
            win_sb, s2h_sb, outp_sb, cpow_sb = [], [], [], []
            for i in range(NB):
                win_sb.append(wts.tile([P, 4 * P], f32r, name=f"win{i}"))
                nc.sync.dma_start(win_sb[i][:], win_in[i].bitcast(f32r))
                s2h_sb.append(wts.tile([P, 4 * P], f32r, name=f"s2h{i}"))
                nc.sync.dma_start(s2h_sb[i][:], s2h_in[i].bitcast(f32r))
                outp_sb.append(wts.tile([P, 4 * P], f32r, name=f"outp{i}"))
                nc.sync.dma_start(outp_sb[i][:], outp_in[i].bitcast(f32r))
                cpow_sb.append(wts.tile([P, 2, T], f32, name=f"cpow{i}"))
                nc.sync.dma_start(cpow_sb[i][:], cpow_in[i].rearrange("a p t -> p a t"))
            # projection weights last — only needed after the backbone
            outwt_sb = [wts.tile([P, V], bf16, name=f"outwt{d}") for d in range(2)]
            for d in range(2):
                nc.sync.dma_start(outwt_sb[d][:], outwt_in[d])
            if use_outb:
                outb_sb = cst.tile([1, V], bf16)
                nc.sync.dma_start(outb_sb[:], outb_in)
                ones1_sb = cst.tile([1, P], bf16)
                nc.sync.dma_start(ones1_sb[:], onesT[:, :P])

            # ---- embedding gather + pos add ----
            h = wk.tile([P, NT, D], f32)
            for ct in range(NT):
                nc.gpsimd.indirect_dma_start(
                    out=h[:, ct, :], out_offset=None, in_=tok_tab,
                    in_offset=bass.IndirectOffsetOnAxis(ap=ti_sb[:, ct:ct + 1], axis=0))
            nc.vector.tensor_tensor(
                h[:].rearrange("p a b -> p (a b)"), h[:].rearrange("p a b -> p (a b)"),
                pos_sb[:], op=OP.add)

            hsT = [None, None]

            # ---- 4 sequential layer passes ----
            for i in range(NB):
                last = i == NB - 1
                # 1. layernorm stats + z = (h - mean) * rstd   (z in f32r)
                z = lay.tile([P, NT, D], f32r, tag="z")
                stats = lay.tile([P, NT, 6], f32, tag="stats")
                aggr = lay.tile([P, NT, 2], f32, tag="aggr")
                rstd = lay.tile([P, NT], f32, tag="rstd")
                for ct in range(NT):
                    nc.vector.bn_stats(stats[:, ct, :], h[:, ct, :])
                    nc.vector.bn_aggr(aggr[:, ct, :], stats[:, ct, :])
                nc.vector.tensor_scalar_add(rstd[:], aggr[:, :, 1], 1e-5)
                nc.scalar.activation(rstd[:], rstd[:], AF.Sqrt)
                # prefetch the gelu table while the LN chain continues on DVE
                nc.scalar.activation(scrap[:], rstd[0:1, 0:1], AF.Gelu_apprx_tanh)
                nc.vector.reciprocal(rstd[:], rstd[:])
                for ct in range(NT):
                    nc.vector.tensor_scalar(
                        z[:, ct, :], h[:, ct, :], aggr[:, ct, 0:1], rstd[:, ct:ct + 1],
                        op0=OP.subtract, op1=OP.mult)
                # 2. transpose z -> zT [D part, T free]
                zT = [lay.tile([P, T], f32r, tag=f"zT{d}", name=f"zT{d}") for d in range(2)]
                for d in range(2):
                    zt_ps = ps.tile([P, T], f32r, tag="pp", bufs=8, name=f"zt_ps{d}")
                    for ct in range(NT):
                        nc.tensor.transpose(zt_ps[:, ct * P:(ct + 1) * P],
                                            z[:, ct, d * P:(d + 1) * P], id_r[:])
                    nc.scalar.activation(zT[d][:], zt_ps[:], AF.Identity)
                # 3. u = W_in' @ z (+ubias via ones-row matmul); local scan from PSUM
                ns1 = [lay.tile([P, T], f32r, tag=f"ns1{st}", name=f"ns1{st}")
                       for st in range(2)]
                last2 = lay.tile([P, 2], f32r, tag="last2")
                u_ps = [None, None]
                for st in range(2):
                    u_ps[st] = ps.tile([P, T], f32, tag="pp", bufs=8, name=f"u_ps{st}")
                    for kt in range(2):
                        nc.tensor.matmul(u_ps[st][:],
                                         win_sb[i][:, (kt * 2 + st) * P:(kt * 2 + st + 1) * P],
                                         zT[kt][:], start=(kt == 0), stop=False)
                    nc.tensor.matmul(u_ps[st][:],
                                     ub_sb[:, (i * 2 + st) * P:(i * 2 + st + 1) * P],
                                     onesT_sb[:], start=False, stop=True)
                    cb = coef_sb[:, i * 2 + st:i * 2 + st + 1].to_broadcast((P, T))
                    nc.vector.tensor_tensor_scan(ns1[st][:], cb, u_ps[st][:], 0.0,
                                                 op0=OP.mult, op1=OP.add)
                    nc.vector.tensor_copy(last2[:, st:st + 1], ns1[st][:, T - 1:T])
                # 4. export chunk-final states (transposed: 2 descriptors),
                #    AllGather, combine into carry
                lt_ps = ps.tile([2, P], f32r, tag="pp", bufs=8, name="lt_ps")
                nc.tensor.transpose(lt_ps[:], last2[:], id_r[:])
                exp_sb = lay.tile([2, P], f32, tag="exp")
                nc.scalar.activation(exp_sb[:], lt_ps[:], AF.Identity)
                ag_in = dram.tile([2, P], f32, name=f"ag_in{i}")
                ag_out = dram.tile([NC, 2, P], f32, name=f"ag_out{i}",
                                   addr_space="Shared")
                nc.scalar.dma_start(ag_in[:], exp_sb[:])
                nc.gpsimd.collective_compute(
                    "AllGather", OP.bypass, replica_groups=[list(range(NC))],
                    ins=[ag_in[:]], outs=[ag_out[:]])
                gath = lay.tile([8, S], f32, tag="gath")
                nc.scalar.dma_start(gath[:], ag_out[:].rearrange("c a b -> c (a b)"))
                q = lay.tile([8, S], f32, tag="q")
                nc.vector.tensor_tensor(q[:], wm_sb[:, i * S:(i + 1) * S], gath[:],
                                        op=OP.mult)
                c_ps = [None, None]
                A = [lay.tile([P, T], f32r, tag=f"A{st}", name=f"A{st}")
                     for st in range(2)]
                for st in range(2):
                    c_ps[st] = ps.tile([P, 1], f32, tag="pp", bufs=8, name=f"c_ps{st}")
                    nc.tensor.matmul(c_ps[st][:], q[:, st * P:(st + 1) * P],
                                     ones8_sb[:], start=True, stop=True)
                    nc.vector.tensor_scalar_mul(A[st][:], cpow_sb[i][:, st, :],
                                                c_ps[st][:, 0:1])
                # 5. mixed = gelu(s2h @ (ns1 + A) + dprime*z + dbias)
                #    (the ns1 matmuls are issued first: they overlap the collective)
                mixed = [lay.tile([P, T], f32r, tag=f"mix{d}", name=f"mix{d}")
                         for d in range(2)]
                gin = lay.tile([P, T], f32, tag="gin")
                m_ps = [None, None]
                for d in range(2):
                    m_ps[d] = ps.tile([P, T], f32, tag="pp", bufs=8, name=f"m_ps{d}")
                    for st in range(2):
                        nc.tensor.matmul(m_ps[d][:],
                                         s2h_sb[i][:, (st * 2 + d) * P:(st * 2 + d + 1) * P],
                                         ns1[st][:], start=(st == 0), stop=False)
                for d in range(2):
                    for st in range(2):
                        nc.tensor.matmul(m_ps[d][:],
                                         s2h_sb[i][:, (st * 2 + d) * P:(st * 2 + d + 1) * P],
                                         A[st][:], start=False, stop=(st == 1))
                    nc.vector.scalar_tensor_tensor(
                        gin[:], zT[d][:].bitcast(f32),
                        dp_sb[:, i * 2 + d:i * 2 + d + 1], m_ps[d][:],
                        op0=OP.mult, op1=OP.add)
                    nc.scalar.activation(mixed[d][:], gin[:], AF.Gelu_apprx_tanh,
                                         bias=db_sb[:, i * 2 + d:i * 2 + d + 1])
                # prefetch sqrt table for the next layer's LN during the tail
                if not last:
                    nc.scalar.activation(scrap[:], mixed[1][0:1, 0:1], AF.Sqrt)
                # 6. delta = outp_W' @ mixed (+outp_b)
                delta = [lay.tile([P, T], f32r, tag=f"del{d}", name=f"del{d}")
                         for d in range(2)]
                for d2 in range(2):
                    d_ps = ps.tile([P, T], f32, tag="pp", bufs=8, name=f"d_ps{d2}")
                    for d in range(2):
                        nc.tensor.matmul(d_ps[:],
                                         outp_sb[i][:, (d * 2 + d2) * P:(d * 2 + d2 + 1) * P],
                                         mixed[d][:], start=(d == 0), stop=(d == 1))
                    nc.scalar.activation(delta[d2][:], d_ps[:], AF.Identity,
                                         bias=ob_sb[:, i * 2 + d2:i * 2 + d2 + 1])
                # 7. residual
                if not last:
                    for ct in range(NT):
                        dT_ps = ps.tile([P, D], f32r, tag="pp", bufs=8, name="dT_ps")
                        for d2 in range(2):
                            nc.tensor.transpose(dT_ps[:, d2 * P:(d2 + 1) * P],
                                                delta[d2][:, ct * P:(ct + 1) * P], id_r[:])
                        nc.vector.tensor_tensor(h[:, ct, :], h[:, ct, :],
                                                dT_ps[:].bitcast(f32), op=OP.add)
                else:
                    # hsT = h^T + delta in [D part, T free], bf16 for projection
                    for d2 in range(2):
                        hT_ps = ps.tile([P, T], f32, tag="pp", bufs=8, name=f"hT_ps{d2}")
                        for ct in range(NT):
                            nc.tensor.transpose(hT_ps[:, ct * P:(ct + 1) * P],
                                                h[:, ct, d2 * P:(d2 + 1) * P], id_f[:])
                        hsT[d2] = wk.tile([P, T], bf16, name=f"hsT{d2}")
                        nc.vector.tensor_tensor(hsT[d2][:], delta[d2][:].bitcast(f32),
                                                hT_ps[:], op=OP.add)

            # ---- output projection: out[t, v] = hsT[:, t] . outwt[:, v] ----
            for mt in range(NT):
                for vc in range(NVC):
                    p_ps = ps.tile([P, VC], f32, tag="pp", bufs=8, name="p_ps")
                    for d in range(2):
                        nc.tensor.matmul(p_ps[:], hsT[d][:, mt * P:(mt + 1) * P],
                                         outwt_sb[d][:, vc * VC:(vc + 1) * VC],
                                         start=(d == 0),
                                         stop=(d == 1 and not use_outb))
                    if use_outb:
                        nc.tensor.matmul(p_ps[:], ones1_sb[:],
                                         outb_sb[:, vc * VC:(vc + 1) * VC],
                                         start=False, stop=True)
                    st_t = stg.tile([P, VC], f32, tag="stg", bufs=8)
                    if vc % 2 == 0:
                        nc.vector.tensor_copy(st_t[:], p_ps[:])
                    else:
                        nc.scalar.activation(st_t[:], p_ps[:], AF.Identity)
                    nc.sync.dma_start(
                        out_d[mt * P:(mt + 1) * P, vc * VC:(vc + 1) * VC], st_t[:])

    nc.compile()
    _cache[use_outb] = nc
    return nc


def _pack_lhsT(w):
    """w: [M, K] weight for out = w @ x. Returns [128, (K/128)*(M/128)*128] lhsT pack;
    block b = kt*nmt + mt holds lhsT[kt*128+p, mt*128+m]."""
    M, K = w.shape
    lhsT = np.ascontiguousarray(w.T)                       # [K, M]
    t = lhsT.reshape(K // P, P, M // P, P)                 # [kt, p, mt, m]
    return np.ascontiguousarray(t.transpose(1, 0, 2, 3).reshape(P, -1))


def kernel(**inputs):
    xs = {k: np.asarray(v) for k, v in inputs.items()}
    tokens = xs["tokens"].astype(np.int32)
    token_embed = xs["token_embed"].astype(np.float32)
    pos_embed = xs["pos_embed"].astype(np.float32)
    in_to_state = xs["in_to_state"].astype(np.float64)
    state_to_hidden = xs["state_to_hidden"].astype(np.float64)
    direct = xs["direct"].astype(np.float64)
    a_diag = xs["a_diag"].astype(np.float64)
    g_diag = xs["g_diag"].astype(np.float64)
    dtp = xs["dt"].astype(np.float64)
    ln_w = xs["ln_w"].astype(np.float64)
    ln_b = xs["ln_b"].astype(np.float64)
    outp_W = xs["outp_W"].astype(np.float64)
    outp_b = xs["outp_b"].astype(np.float32)
    out_W = xs["out_W"].astype(np.float32)
    out_b = xs["out_b"].astype(np.float32)

    def softplus(x):
        return np.logaddexp(0.0, x)

    dt_e = softplus(dtp) + 1e-4
    coeff = np.exp(-softplus(g_diag) * dt_e) * np.cos(a_diag * dt_e)   # [NB, S]
    cdecay = coeff ** T                                                 # [NB, S]
    # c^(t+1) tables for the carry correction, [NB, 2, P, T]
    tpow = np.arange(1, T + 1, dtype=np.float64)
    cpow = coeff.reshape(NB, 2, P, 1) ** tpow.reshape(1, 1, 1, T)

    # packed weights (shared across cores)
    win_pack = np.stack([_pack_lhsT(in_to_state[i] * ln_w[i][None, :]) for i in range(NB)]).astype(np.float32)
    s2h_pack = np.stack([_pack_lhsT(state_to_hidden[i]) for i in range(NB)]).astype(np.float32)
    outp_pack = np.stack([_pack_lhsT(outp_W[i]) for i in range(NB)]).astype(np.float32)
    outwt_pack = np.ascontiguousarray(out_W.T.reshape(2, P, V))
    import ml_dtypes
    outwt_bf16 = outwt_pack.astype(ml_dtypes.bfloat16)
    ubias = np.stack([in_to_state[i] @ ln_b[i] for i in range(NB)])     # [NB, S]
    dprime = direct * ln_w                                              # [NB, D]
    dbias = direct * ln_b                                               # [NB, D]

    def cols(v):  # [NB, 256] -> [128, NB*2] with col (i*2+half)
        return np.ascontiguousarray(
            v.reshape(NB, 2, P).transpose(2, 0, 1).reshape(P, NB * 2)).astype(np.float32)

    use_outb = bool(np.any(out_b != 0.0))

    base = dict(
        tok_tab=token_embed, ident=np.eye(P, dtype=np.float32),
        ones8=np.ones((8, 1), np.float32),
        onesT=np.ones((1, T), ml_dtypes.bfloat16),
        coef_in=cols(coeff),
        ubias_in=ubias.reshape(1, NB * 2 * P).astype(ml_dtypes.bfloat16),
        dprime_in=cols(dprime), dbias_in=cols(dbias),
        opb_in=cols(np.broadcast_to(outp_b, (NB, D)).astype(np.float64)),
        win_in=win_pack, s2h_in=s2h_pack, outp_in=outp_pack,
        cpow_in=cpow.astype(np.float32),
        outwt_in=outwt_bf16, outb_in=out_b.reshape(1, V).astype(ml_dtypes.bfloat16),
    )

    in_maps = []
    for k in range(NC):
        sl = slice(k * T, (k + 1) * T)
        tk = tokens[sl].reshape(NT, P).T.copy()            # [128, NT]
        pos = np.ascontiguousarray(
            pos_embed[sl].reshape(NT, P, D).transpose(1, 0, 2).reshape(P, NT * D))
        # carry weights: wmat[j, s] = cdecay[s]^(k-1-j) for j<k else 0
        wm = np.zeros((8, NB, S), np.float64)
        for j in range(k):
            wm[j] = cdecay ** (k - 1 - j)
        wm = wm.reshape(8, NB * S).astype(np.float32)
        in_maps.append(dict(base, tok_idx=tk, pos_pre=pos, wmat_in=wm))

    nc = _build(use_outb)
    trace = bool(os.environ.get("BASS_KERNEL_TRACE"))
    res = run_bass_kernel_spmd(nc, in_maps, core_ids=list(range(NC)), trace=trace)
    if trace:
        kernel.last_exec_time_ns = res.exec_time_ns
        kernel.last_results = res
    return np.concatenate([res.results[k]["out"] for k in range(NC)], axis=0)
